# revision 9
# baseline (speedup 1.0000x reference)
"""Trainium2 Bass kernel for the CMA momentum-memory update (nn_CMA_52956946760162).

Strategy (class-sharded, present-only compact packing):
- Shard the C=4096 classes across 8 cores (512 classes/core), no collectives.
- Host packs, per (core, modality), the *present* (label,cam) segments and
  present labels into chunks of <=128 one-hot columns / <=128*B feature rows
  (whole classes per chunk). The one-hot entries are pre-scaled with the
  momentum/count coefficients (b_c = sigma_or_1/cnt, b_g = sigma/cnt), and a
  segment column and its class column share the same matmul, so one tensor-
  engine pass produces both per-(label,cam) and per-label scaled sums in PSUM.
- Host gathers the corresponding memory-bank rows densely (mem_in), so every
  device DMA is a dense [128 x 2048] f32 block. The device computes
  out = a * mem + psum in a single fused DVE op per chunk and streams it out.
- Rows absent from the batch leave memory unchanged; the host passes them
  through directly from the input banks during output assembly and scatters
  the device-computed rows over them.
"""

import numpy as np

C, K, D, N = 4096, 6, 2048, 16384
SIGMA = 0.2
M = 8                 # cores
CPC = C // M          # classes per core = 512
CK = C * K
F32 = np.float32

_BUILD_CACHE = {}


def _pack_core_modality(core, feats, labels, cams, valid, B, nch, mcap):
    """Pack one (core, modality) into chunk tensors.

    Columns with a != 0 (need a memory-bank row for the blend) are packed into
    positions [0, mcap); columns with a == 0 (present-but-invalid segments,
    whose output is the raw scaled sum) go to positions [mcap, 128). The
    device only DMAs mem rows for the first mcap positions.

    Returns fpad [nch*B*128, D], oh [nch, B*128, 128], avec [128, nch],
    mem_idx [nch, mcap] (merged row id: class c -> c, seg s -> CPC + s,
    pad -> -1), out_idx [nch, 128] (same id scheme, all used columns).
    """
    c0 = core * CPC
    mask = (labels >= c0) & (labels < c0 + CPC)
    rows_all = np.nonzero(mask)[0]
    lab = labels[rows_all] - c0
    seg = lab * K + cams[rows_all]
    order = np.argsort(seg, kind="stable")
    rows_all, lab, seg = rows_all[order], lab[order], seg[order]

    ccnt = np.bincount(seg, minlength=CPC * K).astype(F32)
    gcnt = np.bincount(lab, minlength=CPC).astype(F32)
    v = np.asarray(valid[c0:c0 + CPC]).reshape(CPC * K)
    a_c = np.where(v, 1.0 - SIGMA, 0.0).astype(F32)
    b_c = (np.where(v, SIGMA, 1.0) / np.maximum(ccnt, 1.0)).astype(F32)
    b_g = (SIGMA / np.maximum(gcnt, 1.0)).astype(F32)

    cpres = ccnt > 0
    class_start = np.searchsorted(lab, np.arange(CPC + 1))
    nseg_per_class = cpres.reshape(CPC, K).sum(axis=1)

    chunk_id = np.empty(len(rows_all), np.int64)
    slot = np.empty(len(rows_all), np.int64)
    segcol_of = np.empty(CPC * K, np.int64)
    ccol_of = np.empty(CPC, np.int64)
    mem_idx = np.full((nch, mcap), -1, np.int64)
    out_idx = np.full((nch, 128), -1, np.int64)
    avec = np.zeros((128, nch), F32)

    # per-class column demand: a!=0 cols (valid present segs + the class col),
    # a==0 cols (invalid present segs)
    vseg = (cpres & v).reshape(CPC, K).sum(axis=1)
    n1_of = vseg + 1
    n0_of = nseg_per_class - vseg

    present = np.nonzero(gcnt > 0)[0]
    chunk_classes = []
    cur, c1, c0n, rws = [], 0, 0, 0
    for c in present:
        nr = int(class_start[c + 1] - class_start[c])
        if cur and (c1 + n1_of[c] > mcap or c0n + n0_of[c] > 128 - mcap
                    or rws + nr > B * 128):
            chunk_classes.append(cur)
            cur, c1, c0n, rws = [], 0, 0, 0
        cur.append(c)
        c1 += int(n1_of[c])
        c0n += int(n0_of[c])
        rws += nr
    if cur:
        chunk_classes.append(cur)
    assert len(chunk_classes) <= nch

    for j, cls_list in enumerate(chunk_classes):
        lo, hi, rws = 0, mcap, 0     # a!=0 cols from 0 up; a==0 cols from mcap up
        for c in cls_list:
            segs_c = np.nonzero(cpres[c * K:(c + 1) * K])[0] + c * K
            for s in segs_c:
                if a_c[s] != 0.0:
                    p, lo = lo, lo + 1
                else:
                    p, hi = hi, hi + 1
                segcol_of[s] = p
                out_idx[j, p] = CPC + s
                avec[p, j] = a_c[s]
                if p < mcap:
                    mem_idx[j, p] = CPC + s
            p, lo = lo, lo + 1
            ccol_of[c] = p
            out_idx[j, p] = c
            avec[p, j] = 1.0 - SIGMA
            mem_idx[j, p] = c
            r0, r1 = int(class_start[c]), int(class_start[c + 1])
            chunk_id[r0:r1] = j
            slot[r0:r1] = rws + np.arange(r1 - r0)
            rws += r1 - r0
        assert lo <= mcap and hi <= 128

    fpad = np.zeros((nch, B * 128, D), F32)
    oh = np.zeros((nch, B * 128, 128), F32)
    fpad[chunk_id, slot] = feats[rows_all]
    oh[chunk_id, slot, segcol_of[seg]] = b_c[seg]
    oh[chunk_id, slot, ccol_of[lab]] = b_g[lab]
    return dict(fpad=fpad.reshape(nch * B * 128, D), oh=oh, avec=avec,
                mem_idx=mem_idx, out_idx=out_idx)


def _chunk_stats(labels, cams, valid):
    """Per core: (max rows per class, gcnt, n1_of, n0_of)."""
    out = []
    for core in range(M):
        c0 = core * CPC
        mask = (labels >= c0) & (labels < c0 + CPC)
        lab = labels[mask] - c0
        seg = lab * K + cams[mask]
        gcnt = np.bincount(lab, minlength=CPC)
        cpres = np.bincount(seg, minlength=CPC * K) > 0
        v = np.asarray(valid[c0:c0 + CPC]).reshape(CPC * K)
        vseg = (cpres & v).reshape(CPC, K).sum(axis=1)
        nseg = cpres.reshape(CPC, K).sum(axis=1)
        out.append((int(gcnt.max()), gcnt, vseg + 1, nseg - vseg))
    return out


def _count_chunks(gcnt, n1_of, n0_of, B, mcap):
    j, c1, c0n, rws, any_rows = 0, 0, 0, 0, False
    for c in np.nonzero(gcnt > 0)[0]:
        nr = int(gcnt[c])
        if any_rows and (c1 + n1_of[c] > mcap or c0n + n0_of[c] > 128 - mcap
                         or rws + nr > B * 128):
            j += 1
            c1, c0n, rws = 0, 0, 0
        c1 += int(n1_of[c])
        c0n += int(n0_of[c])
        rws += nr
        any_rows = True
    return j + 1 if any_rows else 0


def _build_program(B, nch, mcap):
    """Build + compile the SPMD Bass program; 2*nch chunks (both modalities)."""
    import concourse.mybir as mybir
    import concourse.tile as tile
    from concourse import bacc

    f32 = mybir.dt.float32
    nc = bacc.Bacc("TRN2", target_bir_lowering=False, debug=False)

    NT = 2 * nch
    fpad = nc.dram_tensor("fpad", [NT * B * 128, D], f32, kind="ExternalInput").ap()
    oh = nc.dram_tensor("oh", [NT, B * 128, 128], f32, kind="ExternalInput").ap()
    memin = nc.dram_tensor("memin", [NT * mcap, D], f32, kind="ExternalInput").ap()
    avec = nc.dram_tensor("avec", [128, NT], f32, kind="ExternalInput").ap()
    out = nc.dram_tensor("out", [NT * 128, D], f32, kind="ExternalOutput").ap()

    with tile.TileContext(nc) as tc:
        with tc.tile_pool(name="const", bufs=1) as constp, \
             tc.tile_pool(name="io", bufs=6) as iop, \
             tc.tile_pool(name="ohp", bufs=5) as ohp, \
             tc.tile_pool(name="ps", bufs=2, space="PSUM") as psp:

            avec_t = constp.tile([128, NT], f32, name="avec_t")
            nc.sync.dma_start(out=avec_t[:], in_=avec[:, :])

            for j in range(NT):
                psum = psp.tile([128, D], f32, tag="ps", name="psum")
                for b in range(B):
                    r0 = (j * B + b) * 128
                    frow = iop.tile([128, D], f32, tag="frow", name="frow")
                    nc.sync.dma_start(out=frow[:], in_=fpad[r0:r0 + 128, :])
                    oht = ohp.tile([128, 128], f32, tag="oh", name="oht")
                    nc.sync.dma_start(out=oht[:], in_=oh[j, b * 128:(b + 1) * 128, :])
                    for t in range(4):
                        sl = slice(t * 512, (t + 1) * 512)
                        nc.tensor.matmul(psum[:, sl], oht[:], frow[:, sl],
                                         start=(b == 0), stop=(b == B - 1))
                mem_sb = iop.tile([128, D], f32, tag="mem", name="mem_sb")
                nc.sync.dma_start(out=mem_sb[:mcap, :], in_=memin[j * mcap:(j + 1) * mcap, :])
                out_sb = iop.tile([128, D], f32, tag="out", name="out_sb")
                nc.vector.scalar_tensor_tensor(
                    out=out_sb[:], in0=mem_sb[:], scalar=avec_t[:, j:j + 1],
                    in1=psum[:], op0=mybir.AluOpType.mult, op1=mybir.AluOpType.add)
                nc.sync.dma_start(out=out[j * 128:(j + 1) * 128, :], in_=out_sb[:])

    nc.compile()
    return nc


def prepare(inputs):
    """Build (or reuse) the program and the per-core input maps + scatter metadata."""
    a = {k: np.ascontiguousarray(np.asarray(v)) for k, v in inputs.items()}
    mods = [
        (a["rgb_feats"], a["rgb_labels"].astype(np.int64), a["rgb_cams"].astype(np.int64),
         a["vis_cam_valid"], a["vis_memory"], a["vis_cam_memory"].reshape(CK, D)),
        (a["ir_feats"], a["ir_labels"].astype(np.int64), a["ir_cams"].astype(np.int64),
         a["ir_cam_valid"], a["ir_memory"], a["ir_cam_memory"].reshape(CK, D)),
    ]

    # global B, mcap, chunk count (uniform across cores -> one SPMD program)
    B = 1
    stats = []
    for feats, labels, cams, valid, gmem, cmem in mods:
        st = _chunk_stats(labels, cams, valid)
        stats.append(st)
        for mx, _, _, _ in st:
            B = max(B, int(np.ceil(mx / 128)))
    best = None
    for mcap in (64, 72, 80, 88, 96, 128):
        nch = 1
        for st in stats:
            for _, gcnt, n1_of, n0_of in st:
                nch = max(nch, _count_chunks(gcnt, n1_of, n0_of, B, mcap))
        bytes_per_core = 2 * nch * (B * 128 * (D + 128) + 128 * D + mcap * D) * 4
        if best is None or bytes_per_core < best[0]:
            best = (bytes_per_core, mcap, nch)
    _, mcap, nch = best

    key = (B, nch, mcap)
    if key not in _BUILD_CACHE:
        _BUILD_CACHE[key] = _build_program(B, nch, mcap)
    nc = _BUILD_CACHE[key]

    in_maps, metas = [], []
    for core in range(M):
        c0 = core * CPC
        packs = []
        for m, (feats, labels, cams, valid, gmem, cmem) in enumerate(mods):
            packs.append(_pack_core_modality(core, feats, labels, cams, valid, B, nch, mcap))
        im = {
            "fpad": np.concatenate([p["fpad"] for p in packs], axis=0),
            "oh": np.concatenate([p["oh"] for p in packs], axis=0),
            "avec": np.concatenate([p["avec"] for p in packs], axis=1),
        }
        memin = np.zeros((2 * nch * mcap, D), F32)
        meta = []
        for m, p in enumerate(packs):
            gmem, cmem = mods[m][4], mods[m][5]
            idx = p["mem_idx"].reshape(nch * mcap)
            used = np.nonzero(idx >= 0)[0]
            gidx = idx[used]
            isg = gidx < CPC
            src = np.where(isg, c0 + gidx, core * CPC * K + (gidx - CPC))
            block = memin[m * nch * mcap:(m + 1) * nch * mcap]
            block[used[isg]] = gmem[src[isg]]
            block[used[~isg]] = cmem[src[~isg]]
            oidx = p["out_idx"].reshape(nch * 128)
            oused = np.nonzero(oidx >= 0)[0]
            ogidx = oidx[oused]
            oisg = ogidx < CPC
            obase = (C + CK) * m
            tgt = np.where(oisg, obase + c0 + ogidx,
                           obase + C + core * CPC * K + (ogidx - CPC))
            meta.append((oused + m * nch * 128, tgt))
        im["memin"] = memin
        in_maps.append(im)
        metas.append(meta)
    return nc, in_maps, metas, a, mods


def assemble(a, mods, metas, results):
    full = np.concatenate([a["vis_memory"], mods[0][5], a["ir_memory"], mods[1][5]],
                          axis=0).astype(F32, copy=True)
    for core in range(M):
        o = results[core]["out"]
        for used, tgt in metas[core]:
            full[tgt] = o[used]
    return full


def kernel(**inputs):
    from concourse.bass_utils import run_bass_kernel_spmd

    nc, in_maps, metas, a, mods = prepare(inputs)
    res = run_bass_kernel_spmd(nc, in_maps, core_ids=list(range(M)))
    return assemble(a, mods, metas, res.results)
